# revision 23
# baseline (speedup 1.0000x reference)
"""Fused multi-core attention kernel for Trainium2 (Bass/Tile), v2.

Problem: BasicAttention block on x[4, 256, 64, 64]:
    q = Wq x + bq ; k = Wk x + bk ; v = Wv x + bv   (1x1 convs)
    energy = q^T k * IC^-0.5 ; attn = softmax(energy, keys)
    y = gamma * (v @ attn^T) + 2 x

Sharding: 8 cores = (batch b in 0..3) x (query-row half r in 0..1),
2048 query rows per core, flash-style (NxN energy never leaves PSUM).

v2 dataflow (all matmul cost on TRN2 = out-columns x 0.417ns; DR mode
doubles contraction per column, measured 215ns/512-col, 110ns/257-col):
  x8 [128,2,4096] fp8 c-pair layout   <- host-cast DMA (keys = [own|other])
  Q  [128,2048], K [128,4096] fp8     <- DR proj + bias (ACT/DVE conv)
  VT [128,32,257] fp8 = gamma*(X^T Wv), col 256 = 1.0 (ones col)
     bv folded into host-side residual: sum_k attn = 1 => y += gamma*bv
  per 512-query chunk, 16 key-block pairs:
    E^T pair [128,2,512] f32 PSUM = Kblk^T Q       (plain fp8, 2 banks x2)
    P^T = exp(scale*E^T) -> fp8 SBUF: split ACT (table exp) / DVE
         (Schraudolph bit-exp: int8(round(E*A+B)) bitcast fp8, ~3% err)
    U^T[qb] [128,257] += P^T_pair_qb^T VTpair      (fp8 DR, 4 PSUM banks)
      col 256 accumulates S = sum_k P (softmax denom) for free
    y^T[qb] = U^T[:,0:256] * (1/S) + xt2[qb]       (DVE recip + STT)
  y returned as y^T [2048, 256] f32; host transposes + assembles.
"""

import os
import sys

for _p in ("/opt/trn_rl_repo", "/root/.axon_site/_ro/trn_rl_repo"):
    if os.path.isdir(_p) and _p not in sys.path:
        sys.path.append(_p)

import numpy as np
import ml_dtypes

import concourse.bass as bass
import concourse.mybir as mybir
import concourse.tile as tile
from concourse.bass_utils import run_bass_kernel_spmd

F8 = mybir.dt.float8e4
F32 = mybir.dt.float32
I8 = mybir.dt.int8
NPF8 = ml_dtypes.float8_e4m3

B, C, H, W = 4, 256, 64, 64
N = H * W              # 4096 pixels (keys)
IC = C // 2            # 128 inter channels
NCORES = 8
ROWS = N * B // NCORES  # 2048 query rows per core
CHUNK = 512            # query rows per softmax chunk
NCH = ROWS // CHUNK    # 4 chunks
MB = N // 128          # 32 key blocks
NP_ = MB // 2          # 16 key-block pairs per chunk
SCALE = float(IC) ** -0.5
# Schraudolph constants for fp8e4m3: bits ~= E*scale*8/ln2 + (56 - 8*0.043)
A_EXP = SCALE * 8.0 / float(np.log(2.0))
B_EXP = 56.0 - 8.0 * 0.043
# key-block pairs whose exp runs on DVE (bit-trick); rest on ACT
DVE_G = frozenset((2, 5, 8, 11, 14))
DR = mybir.MatmulPerfMode.DoubleRow


def _split_waits(nc):
    """This container's walrus accepts only ONE sync-wait per instruction.
    Hoist extra waits onto single-wait NOPs inserted just before the
    instruction on the same engine (identical stall semantics)."""
    for f in nc.m.functions:
        for b in f.blocks:
            insts = b.instructions
            i = 0
            while i < len(insts):
                inst = insts[i]
                si = inst.sync_info
                if si is not None and len(si.on_wait) > 1:
                    waits = list(si.on_wait)
                    si.on_wait = waits[-1:]
                    for w in waits[:-1]:
                        nop = mybir.InstNoOp(
                            name=f"I-wsplit-{nc.next_id()}",
                            engine=inst.engine,
                            ins=[],
                            outs=[],
                            sync_info=mybir.SyncInfo(on_wait=[w], on_update=[]),
                        )
                        insts.insert(i, nop)
                        i += 1
                i += 1


def _build():
    nc = bass.Bass()

    x8_d = nc.dram_tensor("x8", [128, 2 * N], F8, kind="ExternalInput")
    xt2_d = nc.dram_tensor("xt2", [ROWS, C], F32, kind="ExternalInput")
    wqkv_d = nc.dram_tensor("wqkv8", [128, 2 * (2 * IC + C)], F8,
                            kind="ExternalInput")
    bqk_d = nc.dram_tensor("bqk", [IC, 2], F32, kind="ExternalInput")
    gam_d = nc.dram_tensor("gam", [1, 1], F32, kind="ExternalInput")
    y_d = nc.dram_tensor("y", [ROWS, C], F32, kind="ExternalOutput")

    with tile.TileContext(nc) as tc:
        with (
            tc.tile_pool(name="consts", bufs=1) as consts,
            tc.tile_pool(name="xb", bufs=1) as xbp,
            tc.tile_pool(name="kq", bufs=1) as kqp,
            tc.tile_pool(name="vt", bufs=1) as vtp,
            tc.tile_pool(name="pt", bufs=2) as ptp,
            tc.tile_pool(name="sm", bufs=4) as smp,
            tc.tile_pool(name="outp", bufs=4) as outp,
            tc.tile_pool(name="eg", bufs=2, space="PSUM") as egp,
            tc.tile_pool(name="u0", bufs=1, space="PSUM") as up0,
            tc.tile_pool(name="u1", bufs=1, space="PSUM") as up1,
            tc.tile_pool(name="u2", bufs=1, space="PSUM") as up2,
            tc.tile_pool(name="u3", bufs=1, space="PSUM") as up3,
        ):
            ups = [up0, up1, up2, up3]

            # ---- DMA layout: one combined weight load first (gpsimd),
            # strips split gpsimd/sync, floats + xt2 on the ACT queue ----
            x8 = xbp.tile([128, 2, N], F8, tag="x8")
            x8r = x8_d.rearrange("p (t n) -> p t n", t=2)
            wqkv = consts.tile([128, 2, 2 * IC + C], F8, tag="wqkv")
            nc.gpsimd.dma_start(
                out=wqkv, in_=wqkv_d.rearrange("p (t o) -> p t o", t=2)
            )
            wq = wqkv[:, :, 0:IC]
            wk = wqkv[:, :, IC : 2 * IC]
            wv = wqkv[:, :, 2 * IC : 2 * IC + C]
            bqk = consts.tile([IC, 2], F32, tag="bqk")
            nc.scalar.dma_start(out=bqk, in_=bqk_d[:])
            gamb = consts.tile([128, 1], F32, tag="gamb")
            nc.scalar.dma_start(
                out=gamb, in_=bass.AP(tensor=gam_d, offset=0, ap=[[0, 128], [1, 1]])
            )
            for s in (0, 2):
                sl = slice(s * 1024, (s + 1) * 1024)
                nc.gpsimd.dma_start(out=x8[:, :, sl], in_=x8r[:, :, sl])
            for s in (1, 3):
                sl = slice(s * 1024, (s + 1) * 1024)
                nc.sync.dma_start(out=x8[:, :, sl], in_=x8r[:, :, sl])
            # xt2 on the ACT queue; first needed ~20us in (normalize ch0)
            xt2 = consts.tile([128, ROWS // 128, C], F32, tag="xt2")
            nc.scalar.dma_start(
                out=xt2, in_=xt2_d.rearrange("(qb p) c -> p qb c", p=128)
            )

            # ---- projections, strip-major: Q/K/V of each 1024-col strip
            # as it lands, so the PE never outruns the x8 DMA stream ----
            qbuf = kqp.tile([128, ROWS], F8, tag="qbuf")
            kbuf = kqp.tile([128, N], F8, tag="kbuf")
            vt = vtp.tile([128, MB, C + 1], F8, tag="vt")
            nc.gpsimd.memset(vt[:, :, C : C + 1], 1.0)

            def qk_pair(w, buf, pp, bias, on_act):
                ps = egp.tile([128, 2, 512], F32, tag="eg")
                for j in range(2):
                    sl = slice((2 * pp + j) * 512, (2 * pp + j + 1) * 512)
                    nc.tensor.matmul(ps[:, j, :], w, x8[:, :, sl],
                                     start=True, stop=True, perf_mode=DR)
                dst = buf[:, 2 * pp * 512 : (2 * pp + 2) * 512]
                if on_act:
                    nc.scalar.activation(
                        dst, ps.rearrange("p t n -> p (t n)"),
                        mybir.ActivationFunctionType.Identity,
                        bias=bias, scale=1.0,
                    )
                else:
                    nc.vector.tensor_scalar_add(
                        dst, ps.rearrange("p t n -> p (t n)"), bias
                    )

            def v_pair(g, on_act):
                ps = egp.tile([128, 2, CHUNK], F32, tag="eg")
                for j in range(2):
                    nc.tensor.matmul(
                        ps[:, j, 0:C],
                        x8[:, :, (2 * g + j) * 128 : (2 * g + j + 1) * 128],
                        wv,
                        start=True, stop=True, perf_mode=DR,
                    )
                dst = vt[:, 2 * g : 2 * g + 2, 0:C]
                if on_act:
                    nc.scalar.activation(
                        dst, ps[:, :, 0:C],
                        mybir.ActivationFunctionType.Copy, scale=gamb[:, 0:1],
                    )
                else:
                    nc.vector.tensor_scalar_mul(dst, ps[:, :, 0:C], gamb[:, 0:1])

            for s in range(4):
                if s < 2:
                    qk_pair(wq, qbuf, s, bqk[:, 0:1], on_act=(s == 0))
                qk_pair(wk, kbuf, s, bqk[:, 1:2], on_act=(s % 2 == 1))
                for g in range(4 * s, 4 * s + 4):
                    v_pair(g, on_act=(g % 2 == 0))

            # ---- attention main loop ----
            for ch in range(NCH):
                qs = qbuf[:, ch * CHUNK : (ch + 1) * CHUNK]
                ptb = ptp.tile([128, MB, CHUNK], F8, tag="pt")
                us = [
                    ups[qb].tile([128, C + 1], F32, tag=f"u{qb}", name=f"u{qb}")
                    for qb in range(4)
                ]

                def ut_g(g):
                    pair = slice(2 * g, 2 * g + 2)
                    for qb in range(4):
                        nc.tensor.matmul(
                            us[qb],
                            ptb[:, pair, qb * 128 : (qb + 1) * 128],
                            vt[:, pair, :],
                            start=(g == 0),
                            stop=(g == NP_ - 1),
                            perf_mode=DR,
                        )

                for g in range(NP_):
                    eg = egp.tile([128, 2, CHUNK], F32, tag="eg")
                    for j in range(2):
                        mb = 2 * g + j
                        nc.tensor.matmul(
                            eg[:, j, :],
                            kbuf[:, mb * 128 : (mb + 1) * 128],
                            qs,
                            start=True,
                            stop=True,
                        )
                    pair = slice(2 * g, 2 * g + 2)
                    if g % 2 == 1 and g != NP_ - 1:
                        nc.vector.tensor_scalar(
                            ptb[:, pair, :].bitcast(I8),
                            eg,
                            A_EXP,
                            B_EXP,
                            op0=mybir.AluOpType.mult,
                            op1=mybir.AluOpType.add,
                        )
                    else:
                        nc.scalar.activation(
                            ptb[:, pair, :], eg,
                            mybir.ActivationFunctionType.Exp, scale=SCALE,
                        )
                    if g >= 2:
                        ut_g(g - 2)
                for gt in range(NP_ - 2, NP_):
                    ut_g(gt)

                # ---- normalize + residual + store ----
                if ch == NCH - 1:
                    yq = [nc.sync, nc.scalar, nc.scalar, nc.sync]
                else:
                    yq = [nc.sync, nc.gpsimd, nc.scalar, nc.sync]
                for qb in range(4):
                    sinv = smp.tile([128, 1], F32, tag="sinv")
                    nc.vector.reciprocal(sinv, us[qb][:, C : C + 1])
                    yt = outp.tile([128, C], F32, tag="yt")
                    if qb % 2 == 0 or ch == NCH - 1:
                        nc.vector.scalar_tensor_tensor(
                            yt,
                            us[qb][:, 0:C],
                            sinv[:, 0:1],
                            xt2[:, ch * 4 + qb, :],
                            op0=mybir.AluOpType.mult,
                            op1=mybir.AluOpType.add,
                        )
                    else:
                        yt1 = outp.tile([128, C], F32, tag="yt1")
                        nc.scalar.activation(
                            yt1, us[qb][:, 0:C],
                            mybir.ActivationFunctionType.Copy,
                            scale=sinv[:, 0:1],
                        )
                        nc.gpsimd.tensor_tensor(
                            yt, yt1, xt2[:, ch * 4 + qb, :],
                            op=mybir.AluOpType.add,
                        )
                    yq[qb].dma_start(
                        out=y_d.rearrange("(qb p) c -> p qb c", p=128)[
                            :, ch * 4 + qb, :
                        ],
                        in_=yt,
                    )
    _split_waits(nc)
    return nc


_NC_CACHE = None


def _get_nc():
    global _NC_CACHE
    if _NC_CACHE is None:
        _NC_CACHE = _build()
    return _NC_CACHE


def kernel(x, Wq, bq, Wk, bk, Wv, bv, gamma):
    x = np.asarray(x, dtype=np.float32)
    Wq = np.asarray(Wq, np.float32)
    Wk = np.asarray(Wk, np.float32)
    Wv = np.asarray(Wv, np.float32)
    bq = np.asarray(bq, np.float32)
    bk = np.asarray(bk, np.float32)
    bv = np.asarray(bv, np.float32)
    gam = float(np.asarray(gamma, np.float32).reshape(-1)[0])
    nc = _get_nc()

    def cpair(wT):  # [C, M] -> [128, 2, M] with c = t*128 + p
        m = wT.shape[1]
        return np.ascontiguousarray(
            wT.reshape(2, 128, m).transpose(1, 0, 2)
        ).astype(NPF8)

    wqkv = np.concatenate(
        [cpair(Wq.T), cpair(Wk.T), cpair(Wv.T)], axis=2
    )
    shared = {
        "wqkv8": np.ascontiguousarray(wqkv).reshape(128, -1),
        "bqk": np.ascontiguousarray(np.stack([bq, bk], axis=1)),
        "gam": np.full((1, 1), gam, np.float32),
    }
    xflat = x.reshape(B, C, N)
    in_maps = []
    for core in range(NCORES):
        b, r = divmod(core, 2)
        xb = xflat[b]
        # keys ordered [own row half | other half]
        xcat = np.concatenate(
            [xb[:, r * ROWS : (r + 1) * ROWS], xb[:, (1 - r) * ROWS : (2 - r) * ROWS]],
            axis=1,
        )
        x8 = np.ascontiguousarray(
            xcat.reshape(2, 128, N).transpose(1, 0, 2)
        ).astype(NPF8)
        xt2 = np.ascontiguousarray(
            2.0 * xb[:, r * ROWS : (r + 1) * ROWS].T
        ) + (gam * bv)[None, :]
        in_maps.append({"x8": x8, "xt2": xt2.astype(np.float32), **shared})

    trace = bool(int(os.environ.get("KERNEL_TRACE", "0")))
    res = run_bass_kernel_spmd(
        nc, in_maps, core_ids=list(range(NCORES)), trace=trace
    )
    if trace:
        global LAST_RESULT
        LAST_RESULT = res

    out = np.empty((B, C, N), np.float32)
    for core in range(NCORES):
        b, r = divmod(core, 2)
        out[b][:, r * ROWS : (r + 1) * ROWS] = res.results[core]["y"].T
    return out.reshape(B, C, H, W)


if __name__ == "__main__":
    rng = np.random.default_rng(0)
    x = rng.standard_normal((B, C, H, W), dtype=np.float32)
    s = 0.02
    out = kernel(
        x=x,
        Wq=(rng.standard_normal((IC, C)) * s).astype(np.float32),
        bq=np.zeros(IC, np.float32),
        Wk=(rng.standard_normal((IC, C)) * s).astype(np.float32),
        bk=np.zeros(IC, np.float32),
        Wv=(rng.standard_normal((C, C)) * s).astype(np.float32),
        bv=np.zeros(C, np.float32),
        gamma=np.full(1, 0.1, np.float32),
    )
    print("out", out.shape, out.dtype, float(out.ravel()[0]))


# revision 24
# speedup vs baseline: 1.0181x; 1.0181x over previous
"""Fused multi-core attention kernel for Trainium2 (Bass/Tile), v2.

Problem: BasicAttention block on x[4, 256, 64, 64]:
    q = Wq x + bq ; k = Wk x + bk ; v = Wv x + bv   (1x1 convs)
    energy = q^T k * IC^-0.5 ; attn = softmax(energy, keys)
    y = gamma * (v @ attn^T) + 2 x

Sharding: 8 cores = (batch b in 0..3) x (query-row half r in 0..1),
2048 query rows per core, flash-style (NxN energy never leaves PSUM).

v2 dataflow (all matmul cost on TRN2 = out-columns x 0.417ns; DR mode
doubles contraction per column, measured 215ns/512-col, 110ns/257-col):
  x8 [128,2,4096] fp8 c-pair layout   <- host-cast DMA (keys = [own|other])
  Q  [128,2048], K [128,4096] fp8     <- DR proj + bias (ACT/DVE conv)
  VT [128,32,257] fp8 = gamma*(X^T Wv), col 256 = 1.0 (ones col)
     bv folded into host-side residual: sum_k attn = 1 => y += gamma*bv
  per 512-query chunk, 16 key-block pairs:
    E^T pair [128,2,512] f32 PSUM = Kblk^T Q       (plain fp8, 2 banks x2)
    P^T = exp(scale*E^T) -> fp8 SBUF: split ACT (table exp) / DVE
         (Schraudolph bit-exp: int8(round(E*A+B)) bitcast fp8, ~3% err)
    U^T[qb] [128,257] += P^T_pair_qb^T VTpair      (fp8 DR, 4 PSUM banks)
      col 256 accumulates S = sum_k P (softmax denom) for free
    y^T[qb] = U^T[:,0:256] * (1/S) + xt2[qb]       (DVE recip + STT)
  y returned as y^T [2048, 256] f32; host transposes + assembles.
"""

import os
import sys

for _p in ("/opt/trn_rl_repo", "/root/.axon_site/_ro/trn_rl_repo"):
    if os.path.isdir(_p) and _p not in sys.path:
        sys.path.append(_p)

import numpy as np
import ml_dtypes

import concourse.bass as bass
import concourse.mybir as mybir
import concourse.tile as tile
from concourse.bass_utils import run_bass_kernel_spmd

F8 = mybir.dt.float8e4
F32 = mybir.dt.float32
I8 = mybir.dt.int8
NPF8 = ml_dtypes.float8_e4m3

B, C, H, W = 4, 256, 64, 64
N = H * W              # 4096 pixels (keys)
IC = C // 2            # 128 inter channels
NCORES = 8
ROWS = N * B // NCORES  # 2048 query rows per core
CHUNK = 512            # query rows per softmax chunk
NCH = ROWS // CHUNK    # 4 chunks
MB = N // 128          # 32 key blocks
NP_ = MB // 2          # 16 key-block pairs per chunk
SCALE = float(IC) ** -0.5
# Schraudolph constants for fp8e4m3: bits ~= E*scale*8/ln2 + (56 - 8*0.043)
A_EXP = SCALE * 8.0 / float(np.log(2.0))
B_EXP = 56.0 - 8.0 * 0.043
# key-block pairs whose exp runs on DVE (bit-trick); rest on ACT
DVE_G = frozenset((2, 5, 8, 11, 14))
DR = mybir.MatmulPerfMode.DoubleRow


def _split_waits(nc):
    """This container's walrus accepts only ONE sync-wait per instruction.
    Hoist extra waits onto single-wait NOPs inserted just before the
    instruction on the same engine (identical stall semantics)."""
    for f in nc.m.functions:
        for b in f.blocks:
            insts = b.instructions
            i = 0
            while i < len(insts):
                inst = insts[i]
                si = inst.sync_info
                if si is not None and len(si.on_wait) > 1:
                    waits = list(si.on_wait)
                    si.on_wait = waits[-1:]
                    for w in waits[:-1]:
                        nop = mybir.InstNoOp(
                            name=f"I-wsplit-{nc.next_id()}",
                            engine=inst.engine,
                            ins=[],
                            outs=[],
                            sync_info=mybir.SyncInfo(on_wait=[w], on_update=[]),
                        )
                        insts.insert(i, nop)
                        i += 1
                i += 1


def _build():
    nc = bass.Bass()

    x8_d = nc.dram_tensor("x8", [128, 2 * N], F8, kind="ExternalInput")
    xt2_d = nc.dram_tensor("xt2", [ROWS, C], F32, kind="ExternalInput")
    wqkv_d = nc.dram_tensor("wqkv8", [128, 2 * (2 * IC + C)], F8,
                            kind="ExternalInput")
    bqk_d = nc.dram_tensor("bqk", [IC, 2], F32, kind="ExternalInput")
    gam_d = nc.dram_tensor("gam", [1, 1], F32, kind="ExternalInput")
    y_d = nc.dram_tensor("y", [ROWS, C], F32, kind="ExternalOutput")

    with tile.TileContext(nc) as tc:
        with (
            tc.tile_pool(name="consts", bufs=1) as consts,
            tc.tile_pool(name="xb", bufs=1) as xbp,
            tc.tile_pool(name="kq", bufs=1) as kqp,
            tc.tile_pool(name="vt", bufs=1) as vtp,
            tc.tile_pool(name="pt", bufs=2) as ptp,
            tc.tile_pool(name="sm", bufs=4) as smp,
            tc.tile_pool(name="outp", bufs=4) as outp,
            tc.tile_pool(name="eg", bufs=2, space="PSUM") as egp,
            tc.tile_pool(name="u0", bufs=1, space="PSUM") as up0,
            tc.tile_pool(name="u1", bufs=1, space="PSUM") as up1,
            tc.tile_pool(name="u2", bufs=1, space="PSUM") as up2,
            tc.tile_pool(name="u3", bufs=1, space="PSUM") as up3,
        ):
            ups = [up0, up1, up2, up3]

            # ---- DMA layout: one combined weight load first (gpsimd),
            # strips split gpsimd/sync, floats + xt2 on the ACT queue ----
            x8 = xbp.tile([128, 2, N], F8, tag="x8")
            x8r = x8_d.rearrange("p (t n) -> p t n", t=2)
            wqkv = consts.tile([128, 2, 2 * IC + C], F8, tag="wqkv")
            nc.gpsimd.dma_start(
                out=wqkv, in_=wqkv_d.rearrange("p (t o) -> p t o", t=2)
            )
            wq = wqkv[:, :, 0:IC]
            wk = wqkv[:, :, IC : 2 * IC]
            wv = wqkv[:, :, 2 * IC : 2 * IC + C]
            bqk = consts.tile([IC, 2], F32, tag="bqk")
            nc.scalar.dma_start(out=bqk, in_=bqk_d[:])
            gamb = consts.tile([128, 1], F32, tag="gamb")
            nc.scalar.dma_start(
                out=gamb, in_=bass.AP(tensor=gam_d, offset=0, ap=[[0, 128], [1, 1]])
            )
            for s in (0, 2):
                sl = slice(s * 1024, (s + 1) * 1024)
                nc.gpsimd.dma_start(out=x8[:, :, sl], in_=x8r[:, :, sl])
            for s in (1, 3):
                sl = slice(s * 1024, (s + 1) * 1024)
                nc.sync.dma_start(out=x8[:, :, sl], in_=x8r[:, :, sl])
            # xt2 on the ACT queue; first needed ~20us in (normalize ch0)
            xt2 = consts.tile([128, ROWS // 128, C], F32, tag="xt2")
            nc.scalar.dma_start(
                out=xt2, in_=xt2_d.rearrange("(qb p) c -> p qb c", p=128)
            )

            # ---- projections, strip-major: Q/K/V of each 1024-col strip
            # as it lands, so the PE never outruns the x8 DMA stream ----
            qbuf = kqp.tile([128, ROWS], F8, tag="qbuf")
            kbuf = kqp.tile([128, N], F8, tag="kbuf")
            vt = vtp.tile([128, MB, C + 1], F8, tag="vt")
            nc.gpsimd.memset(vt[:, :, C : C + 1], 1.0)

            def qk_pair(w, buf, pp, bias, on_act):
                ps = egp.tile([128, 2, 512], F32, tag="eg")
                for j in range(2):
                    sl = slice((2 * pp + j) * 512, (2 * pp + j + 1) * 512)
                    nc.tensor.matmul(ps[:, j, :], w, x8[:, :, sl],
                                     start=True, stop=True, perf_mode=DR)
                dst = buf[:, 2 * pp * 512 : (2 * pp + 2) * 512]
                if on_act:
                    nc.scalar.activation(
                        dst, ps.rearrange("p t n -> p (t n)"),
                        mybir.ActivationFunctionType.Identity,
                        bias=bias, scale=1.0,
                    )
                else:
                    nc.vector.tensor_scalar_add(
                        dst, ps.rearrange("p t n -> p (t n)"), bias
                    )

            def v_pair(g, on_act):
                ps = egp.tile([128, 2, CHUNK], F32, tag="eg")
                for j in range(2):
                    nc.tensor.matmul(
                        ps[:, j, 0:C],
                        x8[:, :, (2 * g + j) * 128 : (2 * g + j + 1) * 128],
                        wv,
                        start=True, stop=True, perf_mode=DR,
                    )
                dst = vt[:, 2 * g : 2 * g + 2, 0:C]
                if on_act:
                    nc.scalar.activation(
                        dst, ps[:, :, 0:C],
                        mybir.ActivationFunctionType.Copy, scale=gamb[:, 0:1],
                    )
                else:
                    nc.vector.tensor_scalar_mul(dst, ps[:, :, 0:C], gamb[:, 0:1])

            def make_chunk():
                ptb = ptp.tile([128, MB, CHUNK], F8, tag="pt")
                us = [
                    ups[qb].tile([128, C + 1], F32, tag=f"u{qb}", name=f"u{qb}")
                    for qb in range(4)
                ]
                return ptb, us

            def ut_g(ptb, us, g):
                pair = slice(2 * g, 2 * g + 2)
                for qb in range(4):
                    nc.tensor.matmul(
                        us[qb],
                        ptb[:, pair, qb * 128 : (qb + 1) * 128],
                        vt[:, pair, :],
                        start=(g == 0),
                        stop=(g == NP_ - 1),
                        perf_mode=DR,
                    )

            def eg_slot(ch, ptb, us, g):
                qs = qbuf[:, ch * CHUNK : (ch + 1) * CHUNK]
                eg = egp.tile([128, 2, CHUNK], F32, tag="eg")
                for j in range(2):
                    mb = 2 * g + j
                    nc.tensor.matmul(
                        eg[:, j, :],
                        kbuf[:, mb * 128 : (mb + 1) * 128],
                        qs,
                        start=True,
                        stop=True,
                    )
                pair = slice(2 * g, 2 * g + 2)
                if g % 2 == 1 and g != NP_ - 1:
                    nc.vector.tensor_scalar(
                        ptb[:, pair, :].bitcast(I8),
                        eg,
                        A_EXP,
                        B_EXP,
                        op0=mybir.AluOpType.mult,
                        op1=mybir.AluOpType.add,
                    )
                else:
                    nc.scalar.activation(
                        ptb[:, pair, :], eg,
                        mybir.ActivationFunctionType.Exp, scale=SCALE,
                    )
                if g >= 2:
                    ut_g(ptb, us, g - 2)

            def finish_chunk(ch, ptb, us):
                for gt in range(NP_ - 2, NP_):
                    ut_g(ptb, us, gt)
                if ch == NCH - 1:
                    yq = [nc.sync, nc.scalar, nc.scalar, nc.sync]
                else:
                    yq = [nc.sync, nc.gpsimd, nc.scalar, nc.sync]
                for qb in range(4):
                    sinv = smp.tile([128, 1], F32, tag="sinv")
                    nc.vector.reciprocal(sinv, us[qb][:, C : C + 1])
                    yt = outp.tile([128, C], F32, tag="yt")
                    if qb % 2 == 0 or ch == NCH - 1:
                        nc.vector.scalar_tensor_tensor(
                            yt,
                            us[qb][:, 0:C],
                            sinv[:, 0:1],
                            xt2[:, ch * 4 + qb, :],
                            op0=mybir.AluOpType.mult,
                            op1=mybir.AluOpType.add,
                        )
                    else:
                        yt1 = outp.tile([128, C], F32, tag="yt1")
                        nc.scalar.activation(
                            yt1, us[qb][:, 0:C],
                            mybir.ActivationFunctionType.Copy,
                            scale=sinv[:, 0:1],
                        )
                        nc.gpsimd.tensor_tensor(
                            yt, yt1, xt2[:, ch * 4 + qb, :],
                            op=mybir.AluOpType.add,
                        )
                    yq[qb].dma_start(
                        out=y_d.rearrange("(qb p) c -> p qb c", p=128)[
                            :, ch * 4 + qb, :
                        ],
                        in_=yt,
                    )

            # ---- strip-major projections fused with chunk 0: the PE
            # consumes each x8 strip (proj + ch0 attention) as it lands,
            # staying busy through the DMA-paced prologue ----
            ptb0, us0 = make_chunk()
            for s in range(4):
                if s < 2:
                    qk_pair(wq, qbuf, s, bqk[:, 0:1], on_act=(s == 0))
                qk_pair(wk, kbuf, s, bqk[:, 1:2], on_act=(s % 2 == 1))
                for g in range(4 * s, 4 * s + 4):
                    v_pair(g, on_act=(g % 2 == 0))
                for g in range(4 * s, 4 * s + 4):
                    eg_slot(0, ptb0, us0, g)
            finish_chunk(0, ptb0, us0)

            for ch in range(1, NCH):
                ptb, us = make_chunk()
                for g in range(NP_):
                    eg_slot(ch, ptb, us, g)
                finish_chunk(ch, ptb, us)
    _split_waits(nc)
    return nc


_NC_CACHE = None


def _get_nc():
    global _NC_CACHE
    if _NC_CACHE is None:
        _NC_CACHE = _build()
    return _NC_CACHE


def kernel(x, Wq, bq, Wk, bk, Wv, bv, gamma):
    x = np.asarray(x, dtype=np.float32)
    Wq = np.asarray(Wq, np.float32)
    Wk = np.asarray(Wk, np.float32)
    Wv = np.asarray(Wv, np.float32)
    bq = np.asarray(bq, np.float32)
    bk = np.asarray(bk, np.float32)
    bv = np.asarray(bv, np.float32)
    gam = float(np.asarray(gamma, np.float32).reshape(-1)[0])
    nc = _get_nc()

    def cpair(wT):  # [C, M] -> [128, 2, M] with c = t*128 + p
        m = wT.shape[1]
        return np.ascontiguousarray(
            wT.reshape(2, 128, m).transpose(1, 0, 2)
        ).astype(NPF8)

    wqkv = np.concatenate(
        [cpair(Wq.T), cpair(Wk.T), cpair(Wv.T)], axis=2
    )
    shared = {
        "wqkv8": np.ascontiguousarray(wqkv).reshape(128, -1),
        "bqk": np.ascontiguousarray(np.stack([bq, bk], axis=1)),
        "gam": np.full((1, 1), gam, np.float32),
    }
    xflat = x.reshape(B, C, N)
    in_maps = []
    for core in range(NCORES):
        b, r = divmod(core, 2)
        xb = xflat[b]
        # keys ordered [own row half | other half]
        xcat = np.concatenate(
            [xb[:, r * ROWS : (r + 1) * ROWS], xb[:, (1 - r) * ROWS : (2 - r) * ROWS]],
            axis=1,
        )
        x8 = np.ascontiguousarray(
            xcat.reshape(2, 128, N).transpose(1, 0, 2)
        ).astype(NPF8)
        xt2 = np.ascontiguousarray(
            2.0 * xb[:, r * ROWS : (r + 1) * ROWS].T
        ) + (gam * bv)[None, :]
        in_maps.append({"x8": x8, "xt2": xt2.astype(np.float32), **shared})

    trace = bool(int(os.environ.get("KERNEL_TRACE", "0")))
    res = run_bass_kernel_spmd(
        nc, in_maps, core_ids=list(range(NCORES)), trace=trace
    )
    if trace:
        global LAST_RESULT
        LAST_RESULT = res

    out = np.empty((B, C, N), np.float32)
    for core in range(NCORES):
        b, r = divmod(core, 2)
        out[b][:, r * ROWS : (r + 1) * ROWS] = res.results[core]["y"].T
    return out.reshape(B, C, H, W)


if __name__ == "__main__":
    rng = np.random.default_rng(0)
    x = rng.standard_normal((B, C, H, W), dtype=np.float32)
    s = 0.02
    out = kernel(
        x=x,
        Wq=(rng.standard_normal((IC, C)) * s).astype(np.float32),
        bq=np.zeros(IC, np.float32),
        Wk=(rng.standard_normal((IC, C)) * s).astype(np.float32),
        bk=np.zeros(IC, np.float32),
        Wv=(rng.standard_normal((C, C)) * s).astype(np.float32),
        bv=np.zeros(C, np.float32),
        gamma=np.full(1, 0.1, np.float32),
    )
    print("out", out.shape, out.dtype, float(out.ravel()[0]))


# revision 25
# speedup vs baseline: 1.0326x; 1.0143x over previous
"""Fused multi-core attention kernel for Trainium2 (Bass/Tile), v2.

Problem: BasicAttention block on x[4, 256, 64, 64]:
    q = Wq x + bq ; k = Wk x + bk ; v = Wv x + bv   (1x1 convs)
    energy = q^T k * IC^-0.5 ; attn = softmax(energy, keys)
    y = gamma * (v @ attn^T) + 2 x

Sharding: 8 cores = (batch b in 0..3) x (query-row half r in 0..1),
2048 query rows per core, flash-style (NxN energy never leaves PSUM).

v2 dataflow (all matmul cost on TRN2 = out-columns x 0.417ns; DR mode
doubles contraction per column, measured 215ns/512-col, 110ns/257-col):
  x8 [128,2,4096] fp8 c-pair layout   <- host-cast DMA (keys = [own|other])
  Q  [128,2048], K [128,4096] fp8     <- DR proj + bias (ACT/DVE conv)
  VT [128,32,257] fp8 = gamma*(X^T Wv), col 256 = 1.0 (ones col)
     bv folded into host-side residual: sum_k attn = 1 => y += gamma*bv
  per 512-query chunk, 16 key-block pairs:
    E^T pair [128,2,512] f32 PSUM = Kblk^T Q       (plain fp8, 2 banks x2)
    P^T = exp(scale*E^T) -> fp8 SBUF: split ACT (table exp) / DVE
         (Schraudolph bit-exp: int8(round(E*A+B)) bitcast fp8, ~3% err)
    U^T[qb] [128,257] += P^T_pair_qb^T VTpair      (fp8 DR, 4 PSUM banks)
      col 256 accumulates S = sum_k P (softmax denom) for free
    y^T[qb] = U^T[:,0:256] * (1/S) + xt2[qb]       (DVE recip + STT)
  y returned as y^T [2048, 256] f32; host transposes + assembles.
"""

import os
import sys

for _p in ("/opt/trn_rl_repo", "/root/.axon_site/_ro/trn_rl_repo"):
    if os.path.isdir(_p) and _p not in sys.path:
        sys.path.append(_p)

import numpy as np
import ml_dtypes

import concourse.bass as bass
import concourse.mybir as mybir
import concourse.tile as tile
from concourse.bass_utils import run_bass_kernel_spmd

F8 = mybir.dt.float8e4
F32 = mybir.dt.float32
I8 = mybir.dt.int8
NPF8 = ml_dtypes.float8_e4m3

B, C, H, W = 4, 256, 64, 64
N = H * W              # 4096 pixels (keys)
IC = C // 2            # 128 inter channels
NCORES = 8
ROWS = N * B // NCORES  # 2048 query rows per core
CHUNK = 512            # query rows per softmax chunk
NCH = ROWS // CHUNK    # 4 chunks
MB = N // 128          # 32 key blocks
NP_ = MB // 2          # 16 key-block pairs per chunk
SCALE = float(IC) ** -0.5
# Schraudolph constants for fp8e4m3: bits ~= E*scale*8/ln2 + (56 - 8*0.043)
A_EXP = SCALE * 8.0 / float(np.log(2.0))
B_EXP = 56.0 - 8.0 * 0.043
# key-block pairs whose exp runs on DVE (bit-trick); rest on ACT
DVE_G = frozenset((2, 5, 8, 11, 14))
DR = mybir.MatmulPerfMode.DoubleRow


def _split_waits(nc):
    """This container's walrus accepts only ONE sync-wait per instruction.
    Hoist extra waits onto single-wait NOPs inserted just before the
    instruction on the same engine (identical stall semantics)."""
    for f in nc.m.functions:
        for b in f.blocks:
            insts = b.instructions
            i = 0
            while i < len(insts):
                inst = insts[i]
                si = inst.sync_info
                if si is not None and len(si.on_wait) > 1:
                    waits = list(si.on_wait)
                    si.on_wait = waits[-1:]
                    for w in waits[:-1]:
                        nop = mybir.InstNoOp(
                            name=f"I-wsplit-{nc.next_id()}",
                            engine=inst.engine,
                            ins=[],
                            outs=[],
                            sync_info=mybir.SyncInfo(on_wait=[w], on_update=[]),
                        )
                        insts.insert(i, nop)
                        i += 1
                i += 1


def _build():
    nc = bass.Bass()

    x8_d = nc.dram_tensor("x8", [128, 2 * N], F8, kind="ExternalInput")
    xt2_d = nc.dram_tensor("xt2", [ROWS, C], F32, kind="ExternalInput")
    wqkv_d = nc.dram_tensor("wqkv8", [128, 2 * (2 * IC + C)], F8,
                            kind="ExternalInput")
    bqk_d = nc.dram_tensor("bqk", [IC, 2], F32, kind="ExternalInput")
    gam_d = nc.dram_tensor("gam", [1, 1], F32, kind="ExternalInput")
    y_d = nc.dram_tensor("y", [ROWS, C], F32, kind="ExternalOutput")

    with tile.TileContext(nc) as tc:
        with (
            tc.tile_pool(name="consts", bufs=1) as consts,
            tc.tile_pool(name="xb", bufs=1) as xbp,
            tc.tile_pool(name="kq", bufs=1) as kqp,
            tc.tile_pool(name="vt", bufs=1) as vtp,
            tc.tile_pool(name="pt", bufs=2) as ptp,
            tc.tile_pool(name="sm", bufs=4) as smp,
            tc.tile_pool(name="outp", bufs=4) as outp,
            tc.tile_pool(name="eg", bufs=2, space="PSUM") as egp,
            tc.tile_pool(name="u0", bufs=1, space="PSUM") as up0,
            tc.tile_pool(name="u1", bufs=1, space="PSUM") as up1,
            tc.tile_pool(name="u2", bufs=1, space="PSUM") as up2,
            tc.tile_pool(name="u3", bufs=1, space="PSUM") as up3,
        ):
            ups = [up0, up1, up2, up3]

            # ---- DMA layout: one combined weight load first (gpsimd),
            # strips split gpsimd/sync, floats + xt2 on the ACT queue ----
            x8 = xbp.tile([128, 2, N], F8, tag="x8")
            x8r = x8_d.rearrange("p (t n) -> p t n", t=2)
            wqkv = consts.tile([128, 2, 2 * IC + C], F8, tag="wqkv")
            nc.gpsimd.dma_start(
                out=wqkv, in_=wqkv_d.rearrange("p (t o) -> p t o", t=2)
            )
            wq = wqkv[:, :, 0:IC]
            wk = wqkv[:, :, IC : 2 * IC]
            wv = wqkv[:, :, 2 * IC : 2 * IC + C]
            bqk = consts.tile([IC, 2], F32, tag="bqk")
            nc.scalar.dma_start(out=bqk, in_=bqk_d[:])
            gamb = consts.tile([128, 1], F32, tag="gamb")
            nc.scalar.dma_start(
                out=gamb, in_=bass.AP(tensor=gam_d, offset=0, ap=[[0, 128], [1, 1]])
            )
            strip_q = [nc.gpsimd, nc.sync, nc.scalar]
            for s in range(8):
                sl = slice(s * 512, (s + 1) * 512)
                strip_q[s % 3].dma_start(out=x8[:, :, sl], in_=x8r[:, :, sl])
            # xt2 on the ACT queue; first needed ~20us in (normalize ch0)
            xt2 = consts.tile([128, ROWS // 128, C], F32, tag="xt2")
            nc.scalar.dma_start(
                out=xt2, in_=xt2_d.rearrange("(qb p) c -> p qb c", p=128)
            )

            # ---- projections, strip-major: Q/K/V of each 1024-col strip
            # as it lands, so the PE never outruns the x8 DMA stream ----
            qbuf = kqp.tile([128, ROWS], F8, tag="qbuf")
            kbuf = kqp.tile([128, N], F8, tag="kbuf")
            vt = vtp.tile([128, MB, C + 1], F8, tag="vt")
            nc.gpsimd.memset(vt[:, :, C : C + 1], 1.0)

            def qk_pair(w, buf, pp, bias, on_act):
                ps = egp.tile([128, 2, 512], F32, tag="eg")
                for j in range(2):
                    sl = slice((2 * pp + j) * 512, (2 * pp + j + 1) * 512)
                    nc.tensor.matmul(ps[:, j, :], w, x8[:, :, sl],
                                     start=True, stop=True, perf_mode=DR)
                dst = buf[:, 2 * pp * 512 : (2 * pp + 2) * 512]
                if on_act:
                    nc.scalar.activation(
                        dst, ps.rearrange("p t n -> p (t n)"),
                        mybir.ActivationFunctionType.Identity,
                        bias=bias, scale=1.0,
                    )
                else:
                    nc.vector.tensor_scalar_add(
                        dst, ps.rearrange("p t n -> p (t n)"), bias
                    )

            def v_pair(g, on_act):
                ps = egp.tile([128, 2, CHUNK], F32, tag="eg")
                for j in range(2):
                    nc.tensor.matmul(
                        ps[:, j, 0:C],
                        x8[:, :, (2 * g + j) * 128 : (2 * g + j + 1) * 128],
                        wv,
                        start=True, stop=True, perf_mode=DR,
                    )
                dst = vt[:, 2 * g : 2 * g + 2, 0:C]
                if on_act:
                    nc.scalar.activation(
                        dst, ps[:, :, 0:C],
                        mybir.ActivationFunctionType.Copy, scale=gamb[:, 0:1],
                    )
                else:
                    nc.vector.tensor_scalar_mul(dst, ps[:, :, 0:C], gamb[:, 0:1])

            def make_chunk():
                ptb = ptp.tile([128, MB, CHUNK], F8, tag="pt")
                us = [
                    ups[qb].tile([128, C + 1], F32, tag=f"u{qb}", name=f"u{qb}")
                    for qb in range(4)
                ]
                return ptb, us

            def ut_g(ptb, us, g):
                pair = slice(2 * g, 2 * g + 2)
                for qb in range(4):
                    nc.tensor.matmul(
                        us[qb],
                        ptb[:, pair, qb * 128 : (qb + 1) * 128],
                        vt[:, pair, :],
                        start=(g == 0),
                        stop=(g == NP_ - 1),
                        perf_mode=DR,
                    )

            def eg_slot(ch, ptb, us, g):
                qs = qbuf[:, ch * CHUNK : (ch + 1) * CHUNK]
                eg = egp.tile([128, 2, CHUNK], F32, tag="eg")
                for j in range(2):
                    mb = 2 * g + j
                    nc.tensor.matmul(
                        eg[:, j, :],
                        kbuf[:, mb * 128 : (mb + 1) * 128],
                        qs,
                        start=True,
                        stop=True,
                    )
                pair = slice(2 * g, 2 * g + 2)
                if g % 2 == 1 and g != NP_ - 1:
                    nc.vector.tensor_scalar(
                        ptb[:, pair, :].bitcast(I8),
                        eg,
                        A_EXP,
                        B_EXP,
                        op0=mybir.AluOpType.mult,
                        op1=mybir.AluOpType.add,
                    )
                else:
                    nc.scalar.activation(
                        ptb[:, pair, :], eg,
                        mybir.ActivationFunctionType.Exp, scale=SCALE,
                    )
                if g >= 2:
                    ut_g(ptb, us, g - 2)

            def finish_chunk(ch, ptb, us):
                for gt in range(NP_ - 2, NP_):
                    ut_g(ptb, us, gt)
                if ch == NCH - 1:
                    yq = [nc.sync, nc.scalar, nc.scalar, nc.sync]
                else:
                    yq = [nc.sync, nc.gpsimd, nc.scalar, nc.sync]
                for qb in range(4):
                    sinv = smp.tile([128, 1], F32, tag="sinv")
                    nc.vector.reciprocal(sinv, us[qb][:, C : C + 1])
                    yt = outp.tile([128, C], F32, tag="yt")
                    if qb % 2 == 0 or ch == NCH - 1:
                        nc.vector.scalar_tensor_tensor(
                            yt,
                            us[qb][:, 0:C],
                            sinv[:, 0:1],
                            xt2[:, ch * 4 + qb, :],
                            op0=mybir.AluOpType.mult,
                            op1=mybir.AluOpType.add,
                        )
                    else:
                        yt1 = outp.tile([128, C], F32, tag="yt1")
                        nc.scalar.activation(
                            yt1, us[qb][:, 0:C],
                            mybir.ActivationFunctionType.Copy,
                            scale=sinv[:, 0:1],
                        )
                        nc.gpsimd.tensor_tensor(
                            yt, yt1, xt2[:, ch * 4 + qb, :],
                            op=mybir.AluOpType.add,
                        )
                    yq[qb].dma_start(
                        out=y_d.rearrange("(qb p) c -> p qb c", p=128)[
                            :, ch * 4 + qb, :
                        ],
                        in_=yt,
                    )

            # ---- strip-major projections fused with chunk 0: the PE
            # consumes each x8 strip (proj + ch0 attention) as it lands,
            # staying busy through the DMA-paced prologue ----
            ptb0, us0 = make_chunk()
            for s in range(4):
                if s < 2:
                    qk_pair(wq, qbuf, s, bqk[:, 0:1], on_act=(s == 0))
                qk_pair(wk, kbuf, s, bqk[:, 1:2], on_act=(s % 2 == 1))
                for g in range(4 * s, 4 * s + 4):
                    v_pair(g, on_act=(g % 2 == 0))
                for g in range(4 * s, 4 * s + 4):
                    eg_slot(0, ptb0, us0, g)
            finish_chunk(0, ptb0, us0)

            for ch in range(1, NCH):
                ptb, us = make_chunk()
                for g in range(NP_):
                    eg_slot(ch, ptb, us, g)
                finish_chunk(ch, ptb, us)
    _split_waits(nc)
    return nc


_NC_CACHE = None


def _get_nc():
    global _NC_CACHE
    if _NC_CACHE is None:
        _NC_CACHE = _build()
    return _NC_CACHE


def kernel(x, Wq, bq, Wk, bk, Wv, bv, gamma):
    x = np.asarray(x, dtype=np.float32)
    Wq = np.asarray(Wq, np.float32)
    Wk = np.asarray(Wk, np.float32)
    Wv = np.asarray(Wv, np.float32)
    bq = np.asarray(bq, np.float32)
    bk = np.asarray(bk, np.float32)
    bv = np.asarray(bv, np.float32)
    gam = float(np.asarray(gamma, np.float32).reshape(-1)[0])
    nc = _get_nc()

    def cpair(wT):  # [C, M] -> [128, 2, M] with c = t*128 + p
        m = wT.shape[1]
        return np.ascontiguousarray(
            wT.reshape(2, 128, m).transpose(1, 0, 2)
        ).astype(NPF8)

    wqkv = np.concatenate(
        [cpair(Wq.T), cpair(Wk.T), cpair(Wv.T)], axis=2
    )
    shared = {
        "wqkv8": np.ascontiguousarray(wqkv).reshape(128, -1),
        "bqk": np.ascontiguousarray(np.stack([bq, bk], axis=1)),
        "gam": np.full((1, 1), gam, np.float32),
    }
    xflat = x.reshape(B, C, N)
    in_maps = []
    for core in range(NCORES):
        b, r = divmod(core, 2)
        xb = xflat[b]
        # keys ordered [own row half | other half]
        xcat = np.concatenate(
            [xb[:, r * ROWS : (r + 1) * ROWS], xb[:, (1 - r) * ROWS : (2 - r) * ROWS]],
            axis=1,
        )
        x8 = np.ascontiguousarray(
            xcat.reshape(2, 128, N).transpose(1, 0, 2)
        ).astype(NPF8)
        xt2 = np.ascontiguousarray(
            2.0 * xb[:, r * ROWS : (r + 1) * ROWS].T
        ) + (gam * bv)[None, :]
        in_maps.append({"x8": x8, "xt2": xt2.astype(np.float32), **shared})

    trace = bool(int(os.environ.get("KERNEL_TRACE", "0")))
    res = run_bass_kernel_spmd(
        nc, in_maps, core_ids=list(range(NCORES)), trace=trace
    )
    if trace:
        global LAST_RESULT
        LAST_RESULT = res

    out = np.empty((B, C, N), np.float32)
    for core in range(NCORES):
        b, r = divmod(core, 2)
        out[b][:, r * ROWS : (r + 1) * ROWS] = res.results[core]["y"].T
    return out.reshape(B, C, H, W)


if __name__ == "__main__":
    rng = np.random.default_rng(0)
    x = rng.standard_normal((B, C, H, W), dtype=np.float32)
    s = 0.02
    out = kernel(
        x=x,
        Wq=(rng.standard_normal((IC, C)) * s).astype(np.float32),
        bq=np.zeros(IC, np.float32),
        Wk=(rng.standard_normal((IC, C)) * s).astype(np.float32),
        bk=np.zeros(IC, np.float32),
        Wv=(rng.standard_normal((C, C)) * s).astype(np.float32),
        bv=np.zeros(C, np.float32),
        gamma=np.full(1, 0.1, np.float32),
    )
    print("out", out.shape, out.dtype, float(out.ravel()[0]))
